# revision 3
# baseline (speedup 1.0000x reference)
"""Trainium2 Bass kernel for nn_MessagePassingEdgeModel (v7: ap_gather design).

Reference computation (per edge e):
    h   = concat(x_s[src[e]], x_t[tgt[e]], edge_attr[e], x_u)      # [256]
    z   = leaky_relu(h @ W1 + b1, 0.01)                            # [256]
    y   = z @ W2 + b2                                              # [64]
    out = y * rsqrt(mean(y*y) + eps) * gamma                       # [64]

Distribution: edges are assigned to the 8 cores by the (src-half, tgt-half)
class of their endpoints (class k served by cores {2k, 2k+1}), so each core
addresses 25000 rows of each node table and indices fit in int16.

Gather strategy: both node tables live in SBUF as one [128, 25000*2] uint16
tile — partitions 0:64 hold x_s features (bf16 hi, bf16 lo interleaved),
partitions 64:128 hold x_t.  A single gpsimd ap_gather per 4096-edge group
with per-16-partition index streams (cores 0-3 get src indices, cores 4-7
get tgt) produces ga[128, 2N]: a stride-2 bf16 view of the hi values is the
stacked [xs; xt] K=128 rhs for layer 1 — no DMA gathers, no transposes.

Per 1024-edge tile (feature-major, edges on the moving dim):
  L1   psum[128, 2048] (chunks side by side): per (chunk, 512-seg):
       A-pass  lhsT=[W1_xs; W1_xt] K=128 bf16, rhs = ga hi view
       B-pass  lhsT=[W1_ea; b1'] K=65 bf16, rhs = [eaT; ones row]
       (b1' = b1 + x_u @ W1[192:256] lands in psum via the ones row)
  act  ONE Prelu (alpha=0.01) over [128, 2048] psum -> z bf16
  L2   psum_y[0:64] = W2^T z (2 chunks x 2 segs)
  yb   = y2 + b2 (DVE, psum->sbuf bf16)
  sq   = yb*yb (DVE);  ones-matmul -> psum_y[64:128] = colsum(sq)
  rsq  = Abs_reciprocal_sqrt(ssq/(64 g^2) + eps/g^2) (ACT) -> bf16
  out  = yb * rsq (DVE) -> bf16, DMA out feature-major [64, E].
Host permutes edges per core and un-permutes/up-casts the output.
"""

import numpy as np
import ml_dtypes

BF = ml_dtypes.bfloat16
P = 128
D = 64
MSG = 256
TILE_E = 1024
GROUP = 4096
TPG = GROUP // TILE_E
HALF = 25000
LEAKY = 0.01
N_CORES = 8
EPS = float(np.finfo(np.float32).eps)


def build_nc(t_groups):
    import concourse.bacc as bacc
    import concourse.tile as tile
    from concourse import mybir

    f32 = mybir.dt.float32
    bf16 = mybir.dt.bfloat16
    u16 = mybir.dt.uint16
    i16 = mybir.dt.int16
    AF = mybir.ActivationFunctionType
    OP = mybir.AluOpType
    e_pad = t_groups * GROUP

    nc = bacc.Bacc(None, target_bir_lowering=False, debug=False,
                   num_swdge_queues=4)

    tbl = nc.dram_tensor("tbl", [P, 2 * HALF], u16, kind="ExternalInput")
    idx = nc.dram_tensor("idx", [t_groups, P, GROUP // 16], i16,
                         kind="ExternalInput")
    eaT = nc.dram_tensor("eaT", [D + 1, e_pad], bf16, kind="ExternalInput")
    wa = nc.dram_tensor("wa", [P, MSG], bf16, kind="ExternalInput")
    wb = nc.dram_tensor("wb", [D + 1, MSG], bf16, kind="ExternalInput")
    w2 = nc.dram_tensor("w2", [P, P], bf16, kind="ExternalInput")
    ones64 = nc.dram_tensor("ones64", [D, D], bf16, kind="ExternalInput")
    cst = nc.dram_tensor("cst", [P, 3], f32, kind="ExternalInput")
    outT = nc.dram_tensor("outT", [D, e_pad], bf16, kind="ExternalOutput")

    with tile.TileContext(nc) as tc:
        with (
            nc.allow_low_precision(reason="bf16 matmul path"),
            tc.tile_pool(name="const", bufs=1) as cp,
            tc.tile_pool(name="gb", bufs=2) as gb,
            tc.tile_pool(name="sb", bufs=2) as sb,
            tc.tile_pool(name="ob", bufs=3) as ob,
            tc.tile_pool(name="psl", bufs=1, space="PSUM") as psL,
            tc.tile_pool(name="psy", bufs=2, space="PSUM") as psY,
        ):
            tbl_t = cp.tile([P, 2 * HALF], u16)
            nc.sync.dma_start(tbl_t[:], tbl[:])
            wa_t = cp.tile([P, MSG], bf16)
            nc.sync.dma_start(wa_t[:], wa[:])
            wb_t = cp.tile([D + 1, MSG], bf16)
            nc.sync.dma_start(wb_t[:], wb[:])
            w2_t = cp.tile([P, P], bf16)
            nc.sync.dma_start(w2_t[:], w2[:])
            on_t = cp.tile([D, D], bf16)
            nc.sync.dma_start(on_t[:], ones64[:])
            cst_t = cp.tile([P, 3], f32)
            nc.sync.dma_start(cst_t[:], cst[:])
            b2col = cst_t[0:D, 0:1]
            scl = cst_t[D:P, 1:2]
            bia = cst_t[D:P, 2:3]

            for g in range(t_groups):
                idx_t = gb.tile([P, GROUP // 16], i16, tag="idx")
                nc.sync.dma_start(idx_t[:], idx[g])
                ga = gb.tile([P, 2 * GROUP], u16, tag="ga")
                nc.gpsimd.ap_gather(
                    out_ap=ga[:].rearrange("p (n d) -> p n d", d=2),
                    in_ap=tbl_t[:].rearrange("p (n d) -> p n d", d=2),
                    idxs_ap=idx_t[:],
                    channels=P,
                    num_elems=HALF,
                    d=2,
                    num_idxs=GROUP,
                )
                ga_hi = ga[:].bitcast(bf16).rearrange(
                    "p (n two) -> p two n", two=2)[:, 0, :]
                ea_t = gb.tile([D + 1, GROUP], bf16, tag="ea")
                nc.sync.dma_start(ea_t[:],
                                  eaT[:, g * GROUP:(g + 1) * GROUP])

                for ti in range(TPG):
                    toff = ti * TILE_E
                    e0 = g * GROUP + toff

                    pl1 = psL.tile([P, 2 * TILE_E], mybir.dt.float32,
                                   tag="l1")
                    for c in range(2):
                        for s in range(2):
                            sl = slice(c * TILE_E + s * 512,
                                       c * TILE_E + (s + 1) * 512)
                            rsl = slice(toff + s * 512, toff + (s + 1) * 512)
                            nc.tensor.matmul(
                                pl1[:, sl],
                                lhsT=wa_t[:, c * P:(c + 1) * P],
                                rhs=ga_hi[:, rsl],
                                start=True, stop=False)
                            nc.tensor.matmul(
                                pl1[:, sl],
                                lhsT=wb_t[:, c * P:(c + 1) * P],
                                rhs=ea_t[:, rsl],
                                start=False, stop=True)

                    z = sb.tile([P, 2 * TILE_E], bf16, tag="z")
                    nc.scalar.activation(z[:], pl1[:], AF.Prelu,
                                         bias=0.0, scale=1.0, alpha=LEAKY)

                    py = psY.tile([P, TILE_E], mybir.dt.float32, tag="y")
                    for s in range(2):
                        sl = slice(s * 512, (s + 1) * 512)
                        for c in range(2):
                            nc.tensor.matmul(
                                py[0:D, sl],
                                lhsT=w2_t[:, c * D:(c + 1) * D],
                                rhs=z[:, c * TILE_E + s * 512:
                                      c * TILE_E + (s + 1) * 512],
                                start=(c == 0), stop=(c == 1))

                    yb = sb.tile([D, TILE_E], bf16, tag="yb")
                    nc.vector.tensor_scalar_add(yb[:], py[0:D, :], b2col)
                    sq = sb.tile([D, TILE_E], bf16, tag="sq")
                    nc.vector.scalar_tensor_tensor(
                        out=sq[:], in0=yb[:], scalar=1.0,
                        in1=yb[:], op0=OP.mult, op1=OP.mult)
                    for s in range(2):
                        sl = slice(s * 512, (s + 1) * 512)
                        nc.tensor.matmul(py[D:P, sl], lhsT=on_t[:],
                                         rhs=sq[:, sl],
                                         start=True, stop=True)

                    rsq = sb.tile([D, TILE_E], bf16, tag="rsq")
                    nc.scalar.activation(rsq[:], py[D:P, :],
                                         AF.Abs_reciprocal_sqrt,
                                         bias=bia, scale=scl)
                    ot = ob.tile([D, TILE_E], bf16, tag="ot")
                    nc.vector.scalar_tensor_tensor(
                        out=ot[:], in0=yb[:], scalar=1.0,
                        in1=rsq[:], op0=OP.mult, op1=OP.mult)
                    nc.sync.dma_start(outT[:, e0:e0 + TILE_E], ot[:])

    if not nc.is_finalized():
        nc.finalize()
    return nc


def _pack_hilo_u16(x):
    """[rows, 64] f32 -> [64, 2*rows] u16: feature-major (hi, lo) pairs."""
    xT = np.ascontiguousarray(x.T.astype(np.float32))       # [64, rows]
    hi = xT.astype(BF)
    lo = (xT - hi.astype(np.float32)).astype(BF)
    out = np.empty((D, 2 * x.shape[0]), np.uint16)
    out[:, 0::2] = hi.view(np.uint16)
    out[:, 1::2] = lo.view(np.uint16)
    return out


def _wrap16(v):
    """[n] -> [16, n//16]: idx i at (i%16, i//16)."""
    return np.ascontiguousarray(v.reshape(-1, 16).T)


def prep_shared(x_u, W1, b1, W2, b2, gamma):
    W1 = np.asarray(W1, np.float32)
    W2 = np.asarray(W2, np.float32)
    b1p = (np.asarray(b1, np.float32)
           + np.asarray(x_u, np.float32) @ W1[192:256])
    gamma = np.asarray(gamma, np.float32)
    w2p = np.empty((P, P), np.float32)
    w2p[:, 0:D] = W2[0:P]
    w2p[:, D:P] = W2[P:MSG]
    cst = np.zeros((P, 3), np.float32)
    cst[0:D, 0] = np.asarray(b2, np.float32)
    cst[D:P, 1] = 1.0 / (D * gamma * gamma)
    cst[D:P, 2] = EPS / (gamma * gamma)
    return {
        "wa": np.ascontiguousarray(W1[0:P].astype(BF)),
        "wb": np.ascontiguousarray(
            np.concatenate([W1[P:P + D], b1p[None, :]], 0).astype(BF)),
        "w2": w2p.astype(BF),
        "ones64": np.ones((D, D), BF),
        "cst": cst,
    }


def prep_core(core, eids, src, tgt, ea, xs_half, xt_half, t_groups, shared):
    """eids: int64 edge ids assigned to this core (-1 = pad)."""
    e_pad = t_groups * GROUP
    k = core // 2
    hs, ht = k >> 1, k & 1

    valid = eids >= 0
    eid0 = np.where(valid, eids, 0)
    sv = (src[eid0] - hs * HALF).astype(np.int16)
    tv = (tgt[eid0] - ht * HALF).astype(np.int16)
    sv[~valid] = 0
    tv[~valid] = 0

    idxt = np.empty((t_groups, P, GROUP // 16), np.int16)
    for g in range(t_groups):
        ws = _wrap16(sv[g * GROUP:(g + 1) * GROUP])
        wt = _wrap16(tv[g * GROUP:(g + 1) * GROUP])
        idxt[g, 0:D] = np.tile(ws, (4, 1))
        idxt[g, D:P] = np.tile(wt, (4, 1))

    ea_r = np.where(valid[:, None], ea[eid0], 0).astype(np.float32)
    eaT = np.empty((D + 1, e_pad), BF)
    eaT[0:D] = ea_r.T.astype(BF)
    eaT[D] = BF(1.0)

    tbl = np.concatenate([xs_half[hs], xt_half[ht]], axis=0)

    return {"tbl": tbl, "idx": idxt, "eaT": eaT, **shared}


def assign_edges(src, tgt):
    """Split edges into 8 per-core id lists by (src-half, tgt-half) class."""
    cls = (src >= HALF).astype(np.int64) * 2 + (tgt >= HALF)
    order = np.argsort(cls, kind="stable")
    counts = np.bincount(cls, minlength=4)
    lists = []
    pos = 0
    for k in range(4):
        chunk = order[pos:pos + counts[k]]
        pos += counts[k]
        n0 = (len(chunk) + 1) // 2
        lists.append(chunk[:n0])
        lists.append(chunk[n0:])
    return lists


_CACHE = {}
TRACE = False
LAST_RESULT = None


def kernel(x_s, x_t, edge_index, edge_attr, x_u, W1, b1, W2, b2, gamma):
    global LAST_RESULT
    from concourse.bass_utils import run_bass_kernel_spmd

    src = np.asarray(edge_index[0], np.int64)
    tgt = np.asarray(edge_index[1], np.int64)
    ea = np.asarray(edge_attr, np.float32)
    x_s = np.asarray(x_s, np.float32)
    x_t = np.asarray(x_t, np.float32)
    e_total = src.shape[0]

    lists = assign_edges(src, tgt)
    n_max = max(len(l) for l in lists)
    t_groups = -(-n_max // GROUP)

    key = t_groups
    if key not in _CACHE:
        _CACHE[key] = build_nc(t_groups)
    nc = _CACHE[key]

    shared = prep_shared(x_u, W1, b1, W2, b2, gamma)
    xs_half = [_pack_hilo_u16(x_s[0:HALF]), _pack_hilo_u16(x_s[HALF:2 * HALF])]
    xt_half = [_pack_hilo_u16(x_t[0:HALF]), _pack_hilo_u16(x_t[HALF:2 * HALF])]

    e_pad = t_groups * GROUP
    in_maps = []
    eids_all = []
    for c in range(N_CORES):
        eids = np.full(e_pad, -1, np.int64)
        eids[:len(lists[c])] = lists[c]
        eids_all.append(eids)
        in_maps.append(
            prep_core(c, eids, src, tgt, ea, xs_half, xt_half,
                      t_groups, shared))

    res = run_bass_kernel_spmd(nc, in_maps, list(range(N_CORES)), trace=TRACE)
    LAST_RESULT = res

    out = np.empty((e_total, D), np.float32)
    for c in range(N_CORES):
        eids = eids_all[c]
        valid = eids >= 0
        out[eids[valid]] = res.results[c]["outT"].T.astype(np.float32)[valid]
    return out


# revision 5
# speedup vs baseline: 2.7331x; 2.7331x over previous
"""Trainium2 Bass kernel for nn_MessagePassingEdgeModel (v7: ap_gather design).

Reference computation (per edge e):
    h   = concat(x_s[src[e]], x_t[tgt[e]], edge_attr[e], x_u)      # [256]
    z   = leaky_relu(h @ W1 + b1, 0.01)                            # [256]
    y   = z @ W2 + b2                                              # [64]
    out = y * rsqrt(mean(y*y) + eps) * gamma                       # [64]

Distribution: edges are assigned to the 8 cores by the (src-half, tgt-half)
class of their endpoints (class k served by cores {2k, 2k+1}), so each core
addresses 25000 rows of each node table and indices fit in int16.

Gather strategy: both node tables live in SBUF as one [128, 25000*2] uint16
tile — partitions 0:64 hold x_s features (bf16 hi, bf16 lo interleaved),
partitions 64:128 hold x_t.  A single gpsimd ap_gather per 4096-edge group
with per-16-partition index streams (cores 0-3 get src indices, cores 4-7
get tgt) produces ga[128, 2N]: a stride-2 bf16 view of the hi values is the
stacked [xs; xt] K=128 rhs for layer 1 — no DMA gathers, no transposes.

Per 1024-edge tile (feature-major, edges on the moving dim):
  L1   psum[128, 2048] (chunks side by side): per (chunk, 512-seg):
       A-pass  lhsT=[W1_xs; W1_xt] K=128 bf16, rhs = ga hi view
       B-pass  lhsT=[W1_ea; b1'] K=65 bf16, rhs = [eaT; ones row]
       (b1' = b1 + x_u @ W1[192:256] lands in psum via the ones row)
  act  ONE Prelu (alpha=0.01) over [128, 2048] psum -> z bf16
  L2   psum_y[0:64] = W2^T z (2 chunks x 2 segs)
  yb   = y2 + b2 (DVE, psum->sbuf bf16)
  sq   = yb*yb (DVE);  ones-matmul -> psum_y[64:128] = colsum(sq)
  rsq  = Abs_reciprocal_sqrt(ssq/(64 g^2) + eps/g^2) (ACT) -> bf16
  out  = yb * rsq (DVE) -> bf16, DMA out feature-major [64, E].
Host permutes edges per core and un-permutes/up-casts the output.
"""

import numpy as np
import ml_dtypes

BF = ml_dtypes.bfloat16
P = 128
D = 64
MSG = 256
TILE_E = 1024
GROUP = 4096
TPG = GROUP // TILE_E
HALF = 25000
LEAKY = 0.01
N_CORES = 8
EPS = float(np.finfo(np.float32).eps)


def build_nc(t_groups):
    import concourse.bacc as bacc
    import concourse.tile as tile
    from concourse import mybir

    f32 = mybir.dt.float32
    bf16 = mybir.dt.bfloat16
    u16 = mybir.dt.uint16
    i16 = mybir.dt.int16
    AF = mybir.ActivationFunctionType
    OP = mybir.AluOpType
    e_pad = t_groups * GROUP

    nc = bacc.Bacc(None, target_bir_lowering=False, debug=False,
                   num_swdge_queues=4)

    xsh = nc.dram_tensor("xsh", [HALF, P], bf16, kind="ExternalInput")
    xth = nc.dram_tensor("xth", [HALF, P], bf16, kind="ExternalInput")
    sidx = nc.dram_tensor("sidx", [t_groups, P, GROUP // 16], i16,
                          kind="ExternalInput")
    tidx = nc.dram_tensor("tidx", [t_groups, P, GROUP // 16], i16,
                          kind="ExternalInput")
    eaT = nc.dram_tensor("eaT", [D + 1, e_pad], bf16, kind="ExternalInput")
    wa = nc.dram_tensor("wa", [P, MSG], bf16, kind="ExternalInput")
    wb = nc.dram_tensor("wb", [D + 1, MSG], bf16, kind="ExternalInput")
    w2 = nc.dram_tensor("w2", [P, P], bf16, kind="ExternalInput")
    ones64 = nc.dram_tensor("ones64", [D, D], bf16, kind="ExternalInput")
    identb = nc.dram_tensor("identb", [P, P], bf16, kind="ExternalInput")
    cst = nc.dram_tensor("cst", [P, 3], f32, kind="ExternalInput")
    outT = nc.dram_tensor("outT", [D, e_pad], bf16, kind="ExternalOutput")

    with tile.TileContext(nc) as tc:
        with (
            nc.allow_low_precision(reason="bf16 matmul path"),
            tc.tile_pool(name="const", bufs=1) as cp,
            tc.tile_pool(name="gb", bufs=2) as gb,
            tc.tile_pool(name="sb", bufs=2) as sb,
            tc.tile_pool(name="ob", bufs=3) as ob,
            tc.tile_pool(name="psl", bufs=1, space="PSUM") as psL,
            tc.tile_pool(name="psy", bufs=1, space="PSUM") as psY,
            tc.tile_pool(name="pst", bufs=2, space="PSUM") as psT,
        ):
            wa_t = cp.tile([P, MSG], bf16)
            nc.sync.dma_start(wa_t[:], wa[:])
            wb_t = cp.tile([D + 1, MSG], bf16)
            nc.sync.dma_start(wb_t[:], wb[:])
            w2_t = cp.tile([P, P], bf16)
            nc.sync.dma_start(w2_t[:], w2[:])
            on_t = cp.tile([D, D], bf16)
            nc.sync.dma_start(on_t[:], ones64[:])
            identb_t = cp.tile([P, P], bf16)
            nc.sync.dma_start(identb_t[:], identb[:])
            cst_t = cp.tile([P, 3], f32)
            nc.sync.dma_start(cst_t[:], cst[:])
            b2col = cst_t[0:D, 0:1]
            scl = cst_t[D:P, 1:2]
            bia = cst_t[D:P, 2:3]

            for g in range(t_groups):
                sit = gb.tile([P, GROUP // 16], i16, tag="sit")
                nc.sync.dma_start(sit[:], sidx[g])
                tit = gb.tile([P, GROUP // 16], i16, tag="tit")
                nc.sync.dma_start(tit[:], tidx[g])
                gx = gb.tile([P, GROUP], bf16, tag="gx")
                nc.gpsimd.dma_gather(
                    out_ap=gx[:].rearrange("p (b n) -> p b n", n=P),
                    in_ap=xsh[:],
                    idxs_ap=sit[:],
                    num_idxs=GROUP,
                    num_idxs_reg=GROUP,
                    elem_size=P,
                    transpose=False,
                    single_packet=False,
                    queue_num=(2 * g) % 4,
                )
                gt = gb.tile([P, GROUP], bf16, tag="gt")
                nc.gpsimd.dma_gather(
                    out_ap=gt[:].rearrange("p (b n) -> p b n", n=P),
                    in_ap=xth[:],
                    idxs_ap=tit[:],
                    num_idxs=GROUP,
                    num_idxs_reg=GROUP,
                    elem_size=P,
                    transpose=False,
                    single_packet=False,
                    queue_num=(2 * g + 1) % 4,
                )
                ea_t = gb.tile([D + 1, GROUP], bf16, tag="ea")
                nc.sync.dma_start(ea_t[:],
                                  eaT[:, g * GROUP:(g + 1) * GROUP])

                for ti in range(TPG):
                    toff = ti * TILE_E
                    e0 = g * GROUP + toff

                    ptx = psT.tile([P, TILE_E], bf16, tag="pt", name="ptx")
                    ptt = psT.tile([P, TILE_E], bf16, tag="pt", name="ptt")
                    for j in range(TILE_E // P):
                        bsl = slice((ti * 8 + j) * P, (ti * 8 + j + 1) * P)
                        osl = slice(j * P, (j + 1) * P)
                        nc.tensor.transpose(
                            out=ptx[:, osl], in_=gx[:, bsl],
                            identity=identb_t[:])
                        nc.tensor.transpose(
                            out=ptt[:, osl], in_=gt[:, bsl],
                            identity=identb_t[:])
                    at = sb.tile([P, TILE_E], bf16, tag="at")
                    nc.scalar.activation(at[0:D, :], ptx[0:D, :], AF.Copy)
                    nc.vector.tensor_copy(at[D:P, :], ptt[0:D, :])

                    pl1 = psL.tile([P, 2 * TILE_E], mybir.dt.float32,
                                   tag="l1")
                    for c in range(2):
                        for s in range(2):
                            sl = slice(c * TILE_E + s * 512,
                                       c * TILE_E + (s + 1) * 512)
                            rsl = slice(toff + s * 512, toff + (s + 1) * 512)
                            tsl = slice(s * 512, (s + 1) * 512)
                            nc.tensor.matmul(
                                pl1[:, sl],
                                lhsT=wa_t[:, c * P:(c + 1) * P],
                                rhs=at[:, tsl],
                                start=True, stop=False)
                            nc.tensor.matmul(
                                pl1[:, sl],
                                lhsT=wb_t[:, c * P:(c + 1) * P],
                                rhs=ea_t[:, rsl],
                                start=False, stop=True)

                    z = sb.tile([P, 2 * TILE_E], bf16, tag="z")
                    nc.scalar.activation(z[:], pl1[:], AF.Prelu,
                                         bias=0.0, scale=1.0, alpha=LEAKY)

                    py = psY.tile([P, TILE_E], mybir.dt.float32, tag="y")
                    for s in range(2):
                        sl = slice(s * 512, (s + 1) * 512)
                        for c in range(2):
                            nc.tensor.matmul(
                                py[0:D, sl],
                                lhsT=w2_t[:, c * D:(c + 1) * D],
                                rhs=z[:, c * TILE_E + s * 512:
                                      c * TILE_E + (s + 1) * 512],
                                start=(c == 0), stop=(c == 1))

                    yb = sb.tile([D, TILE_E], bf16, tag="yb")
                    nc.vector.tensor_scalar_add(yb[:], py[0:D, :], b2col)
                    sq = sb.tile([D, TILE_E], bf16, tag="sq")
                    nc.vector.scalar_tensor_tensor(
                        out=sq[:], in0=yb[:], scalar=1.0,
                        in1=yb[:], op0=OP.mult, op1=OP.mult)
                    for s in range(2):
                        sl = slice(s * 512, (s + 1) * 512)
                        nc.tensor.matmul(py[D:P, sl], lhsT=on_t[:],
                                         rhs=sq[:, sl],
                                         start=True, stop=True)

                    rsq = sb.tile([D, TILE_E], bf16, tag="rsq")
                    nc.scalar.activation(rsq[:], py[D:P, :],
                                         AF.Abs_reciprocal_sqrt,
                                         bias=bia, scale=scl)
                    ot = ob.tile([D, TILE_E], bf16, tag="ot")
                    nc.vector.scalar_tensor_tensor(
                        out=ot[:], in0=yb[:], scalar=1.0,
                        in1=rsq[:], op0=OP.mult, op1=OP.mult)
                    nc.sync.dma_start(outT[:, e0:e0 + TILE_E], ot[:])

    if not nc.is_finalized():
        nc.finalize()
    return nc


def _pack_hilo_rows(x):
    """[rows, 64] f32 -> [rows, 128] bf16 (hi | lo) row layout."""
    x = np.asarray(x, np.float32)
    hi = x.astype(BF)
    lo = (x - hi.astype(np.float32)).astype(BF)
    return np.ascontiguousarray(np.concatenate([hi, lo], axis=1))


def _wrap16(v):
    """[n] -> [16, n//16]: idx i at (i%16, i//16)."""
    return np.ascontiguousarray(v.reshape(-1, 16).T)


def prep_shared(x_u, W1, b1, W2, b2, gamma):
    W1 = np.asarray(W1, np.float32)
    W2 = np.asarray(W2, np.float32)
    b1p = (np.asarray(b1, np.float32)
           + np.asarray(x_u, np.float32) @ W1[192:256])
    gamma = np.asarray(gamma, np.float32)
    w2p = np.empty((P, P), np.float32)
    w2p[:, 0:D] = W2[0:P]
    w2p[:, D:P] = W2[P:MSG]
    cst = np.zeros((P, 3), np.float32)
    cst[0:D, 0] = np.asarray(b2, np.float32)
    cst[D:P, 1] = 1.0 / (D * gamma * gamma)
    cst[D:P, 2] = EPS / (gamma * gamma)
    return {
        "wa": np.ascontiguousarray(W1[0:P].astype(BF)),
        "wb": np.ascontiguousarray(
            np.concatenate([W1[P:P + D], b1p[None, :]], 0).astype(BF)),
        "w2": w2p.astype(BF),
        "ones64": np.ones((D, D), BF),
        "identb": np.eye(P, dtype=BF),
        "cst": cst,
    }


def prep_core(core, eids, src, tgt, ea, xs_half, xt_half, t_groups, shared):
    """eids: int64 edge ids assigned to this core (-1 = pad)."""
    e_pad = t_groups * GROUP
    k = core // 2
    hs, ht = k >> 1, k & 1

    valid = eids >= 0
    eid0 = np.where(valid, eids, 0)
    sv = (src[eid0] - hs * HALF).astype(np.int16)
    tv = (tgt[eid0] - ht * HALF).astype(np.int16)
    sv[~valid] = 0
    tv[~valid] = 0

    sidx = np.empty((t_groups, P, GROUP // 16), np.int16)
    tidx = np.empty((t_groups, P, GROUP // 16), np.int16)
    for g in range(t_groups):
        sidx[g] = np.tile(_wrap16(sv[g * GROUP:(g + 1) * GROUP]), (8, 1))
        tidx[g] = np.tile(_wrap16(tv[g * GROUP:(g + 1) * GROUP]), (8, 1))

    ea_r = np.where(valid[:, None], ea[eid0], 0).astype(np.float32)
    eaT = np.empty((D + 1, e_pad), BF)
    eaT[0:D] = ea_r.T.astype(BF)
    eaT[D] = BF(1.0)

    return {"xsh": xs_half[hs], "xth": xt_half[ht],
            "sidx": sidx, "tidx": tidx, "eaT": eaT, **shared}


def assign_edges(src, tgt):
    """Split edges into 8 per-core id lists by (src-half, tgt-half) class."""
    cls = (src >= HALF).astype(np.int64) * 2 + (tgt >= HALF)
    order = np.argsort(cls, kind="stable")
    counts = np.bincount(cls, minlength=4)
    lists = []
    pos = 0
    for k in range(4):
        chunk = order[pos:pos + counts[k]]
        pos += counts[k]
        n0 = (len(chunk) + 1) // 2
        lists.append(chunk[:n0])
        lists.append(chunk[n0:])
    return lists


_CACHE = {}
TRACE = False
LAST_RESULT = None


def kernel(x_s, x_t, edge_index, edge_attr, x_u, W1, b1, W2, b2, gamma):
    global LAST_RESULT
    from concourse.bass_utils import run_bass_kernel_spmd

    src = np.asarray(edge_index[0], np.int64)
    tgt = np.asarray(edge_index[1], np.int64)
    ea = np.asarray(edge_attr, np.float32)
    x_s = np.asarray(x_s, np.float32)
    x_t = np.asarray(x_t, np.float32)
    e_total = src.shape[0]

    lists = assign_edges(src, tgt)
    n_max = max(len(l) for l in lists)
    t_groups = -(-n_max // GROUP)

    key = t_groups
    if key not in _CACHE:
        _CACHE[key] = build_nc(t_groups)
    nc = _CACHE[key]

    shared = prep_shared(x_u, W1, b1, W2, b2, gamma)
    xs_half = [_pack_hilo_rows(x_s[0:HALF]), _pack_hilo_rows(x_s[HALF:2 * HALF])]
    xt_half = [_pack_hilo_rows(x_t[0:HALF]), _pack_hilo_rows(x_t[HALF:2 * HALF])]

    e_pad = t_groups * GROUP
    in_maps = []
    eids_all = []
    for c in range(N_CORES):
        eids = np.full(e_pad, -1, np.int64)
        eids[:len(lists[c])] = lists[c]
        eids_all.append(eids)
        in_maps.append(
            prep_core(c, eids, src, tgt, ea, xs_half, xt_half,
                      t_groups, shared))

    res = run_bass_kernel_spmd(nc, in_maps, list(range(N_CORES)), trace=TRACE)
    LAST_RESULT = res

    out = np.empty((e_total, D), np.float32)
    for c in range(N_CORES):
        eids = eids_all[c]
        valid = eids >= 0
        out[eids[valid]] = res.results[c]["outT"].T.astype(np.float32)[valid]
    return out


# revision 6
# speedup vs baseline: 3.9794x; 1.4560x over previous
"""Trainium2 Bass kernel for nn_MessagePassingEdgeModel (v7: ap_gather design).

Reference computation (per edge e):
    h   = concat(x_s[src[e]], x_t[tgt[e]], edge_attr[e], x_u)      # [256]
    z   = leaky_relu(h @ W1 + b1, 0.01)                            # [256]
    y   = z @ W2 + b2                                              # [64]
    out = y * rsqrt(mean(y*y) + eps) * gamma                       # [64]

Distribution: edges are assigned to the 8 cores by the (src-half, tgt-half)
class of their endpoints (class k served by cores {2k, 2k+1}), so each core
addresses 25000 rows of each node table and indices fit in int16.

Gather strategy: both node tables live in SBUF as one [128, 25000*2] uint16
tile — partitions 0:64 hold x_s features (bf16 hi, bf16 lo interleaved),
partitions 64:128 hold x_t.  A single gpsimd ap_gather per 4096-edge group
with per-16-partition index streams (cores 0-3 get src indices, cores 4-7
get tgt) produces ga[128, 2N]: a stride-2 bf16 view of the hi values is the
stacked [xs; xt] K=128 rhs for layer 1 — no DMA gathers, no transposes.

Per 1024-edge tile (feature-major, edges on the moving dim):
  L1   psum[128, 2048] (chunks side by side): per (chunk, 512-seg):
       A-pass  lhsT=[W1_xs; W1_xt] K=128 bf16, rhs = ga hi view
       B-pass  lhsT=[W1_ea; b1'] K=65 bf16, rhs = [eaT; ones row]
       (b1' = b1 + x_u @ W1[192:256] lands in psum via the ones row)
  act  ONE Prelu (alpha=0.01) over [128, 2048] psum -> z bf16
  L2   psum_y[0:64] = W2^T z (2 chunks x 2 segs)
  yb   = y2 + b2 (DVE, psum->sbuf bf16)
  sq   = yb*yb (DVE);  ones-matmul -> psum_y[64:128] = colsum(sq)
  rsq  = Abs_reciprocal_sqrt(ssq/(64 g^2) + eps/g^2) (ACT) -> bf16
  out  = yb * rsq (DVE) -> bf16, DMA out feature-major [64, E].
Host permutes edges per core and un-permutes/up-casts the output.
"""

import numpy as np
import ml_dtypes

BF = ml_dtypes.bfloat16
P = 128
D = 64
MSG = 256
TILE_E = 1024
GROUP = 4096
TPG = GROUP // TILE_E
HALF = 25000
LEAKY = 0.01
N_CORES = 8
EPS = float(np.finfo(np.float32).eps)


def build_nc(t_groups):
    import concourse.bacc as bacc
    import concourse.tile as tile
    from concourse import mybir

    f32 = mybir.dt.float32
    bf16 = mybir.dt.bfloat16
    u16 = mybir.dt.uint16
    i16 = mybir.dt.int16
    AF = mybir.ActivationFunctionType
    OP = mybir.AluOpType
    e_pad = t_groups * GROUP

    nc = bacc.Bacc(None, target_bir_lowering=False, debug=False,
                   num_swdge_queues=4)

    xsh = nc.dram_tensor("xsh", [HALF, P], bf16, kind="ExternalInput")
    xth = nc.dram_tensor("xth", [HALF, P], bf16, kind="ExternalInput")
    sidx = nc.dram_tensor("sidx", [t_groups, P, GROUP // 16], i16,
                          kind="ExternalInput")
    tidx = nc.dram_tensor("tidx", [t_groups, P, GROUP // 16], i16,
                          kind="ExternalInput")
    eaT = nc.dram_tensor("eaT", [D + 1, e_pad], bf16, kind="ExternalInput")
    wa = nc.dram_tensor("wa", [P, MSG], bf16, kind="ExternalInput")
    wb = nc.dram_tensor("wb", [D + 1, MSG], bf16, kind="ExternalInput")
    w2 = nc.dram_tensor("w2", [P, P], bf16, kind="ExternalInput")
    ones64 = nc.dram_tensor("ones64", [D, D], bf16, kind="ExternalInput")
    identb = nc.dram_tensor("identb", [P, P], bf16, kind="ExternalInput")
    cst = nc.dram_tensor("cst", [P, 3], f32, kind="ExternalInput")
    outT = nc.dram_tensor("outT", [D, e_pad], bf16, kind="ExternalOutput")

    with tile.TileContext(nc) as tc:
        with (
            nc.allow_low_precision(reason="bf16 matmul path"),
            tc.tile_pool(name="const", bufs=1) as cp,
            tc.tile_pool(name="gb", bufs=2) as gb,
            tc.tile_pool(name="sb", bufs=2) as sb,
            tc.tile_pool(name="ob", bufs=3) as ob,
            tc.tile_pool(name="psl", bufs=1, space="PSUM") as psL,
            tc.tile_pool(name="psy", bufs=1, space="PSUM") as psY,
            tc.tile_pool(name="pst", bufs=2, space="PSUM") as psT,
        ):
            wa_t = cp.tile([P, MSG], bf16)
            nc.sync.dma_start(wa_t[:], wa[:])
            wb_t = cp.tile([D + 1, MSG], bf16)
            nc.sync.dma_start(wb_t[:], wb[:])
            w2_t = cp.tile([P, P], bf16)
            nc.sync.dma_start(w2_t[:], w2[:])
            on_t = cp.tile([D, D], bf16)
            nc.sync.dma_start(on_t[:], ones64[:])
            identb_t = cp.tile([P, P], bf16)
            nc.sync.dma_start(identb_t[:], identb[:])
            cst_t = cp.tile([P, 3], f32)
            nc.sync.dma_start(cst_t[:], cst[:])
            b2col = cst_t[0:D, 0:1]
            scl = cst_t[D:P, 1:2]
            bia = cst_t[D:P, 2:3]

            for g in range(t_groups):
                sit = gb.tile([P, GROUP // 16], i16, tag="sit")
                nc.sync.dma_start(sit[:], sidx[g])
                tit = gb.tile([P, GROUP // 16], i16, tag="tit")
                nc.sync.dma_start(tit[:], tidx[g])
                H = GROUP // 2
                gx = gb.tile([P, GROUP], bf16, tag="gx")
                gt = gb.tile([P, GROUP], bf16, tag="gt")
                for h in range(2):
                    nc.gpsimd.dma_gather(
                        out_ap=gx[:, h * H:(h + 1) * H].rearrange(
                            "p (b n) -> p b n", n=P),
                        in_ap=xsh[:],
                        idxs_ap=sit[:, h * (H // 16):(h + 1) * (H // 16)],
                        num_idxs=H,
                        num_idxs_reg=H,
                        elem_size=P,
                        transpose=False,
                        single_packet=False,
                        queue_num=2 * h,
                    )
                    nc.gpsimd.dma_gather(
                        out_ap=gt[:, h * H:(h + 1) * H].rearrange(
                            "p (b n) -> p b n", n=P),
                        in_ap=xth[:],
                        idxs_ap=tit[:, h * (H // 16):(h + 1) * (H // 16)],
                        num_idxs=H,
                        num_idxs_reg=H,
                        elem_size=P,
                        transpose=False,
                        single_packet=False,
                        queue_num=2 * h + 1,
                    )
                ea_t = gb.tile([D + 1, GROUP], bf16, tag="ea")
                nc.sync.dma_start(ea_t[:],
                                  eaT[:, g * GROUP:(g + 1) * GROUP])

                for ti in range(TPG):
                    toff = ti * TILE_E
                    e0 = g * GROUP + toff

                    ptx = psT.tile([P, TILE_E], bf16, tag="pt", name="ptx")
                    ptt = psT.tile([P, TILE_E], bf16, tag="pt", name="ptt")
                    for j in range(TILE_E // P):
                        bsl = slice((ti * 8 + j) * P, (ti * 8 + j + 1) * P)
                        osl = slice(j * P, (j + 1) * P)
                        nc.tensor.transpose(
                            out=ptx[:, osl], in_=gx[:, bsl],
                            identity=identb_t[:])
                        nc.tensor.transpose(
                            out=ptt[:, osl], in_=gt[:, bsl],
                            identity=identb_t[:])
                    at = sb.tile([P, TILE_E], bf16, tag="at")
                    nc.scalar.activation(at[0:D, :], ptx[0:D, :], AF.Copy)
                    nc.vector.tensor_copy(at[D:P, :], ptt[0:D, :])

                    pl1 = psL.tile([P, 2 * TILE_E], mybir.dt.float32,
                                   tag="l1")
                    for c in range(2):
                        for s in range(2):
                            sl = slice(c * TILE_E + s * 512,
                                       c * TILE_E + (s + 1) * 512)
                            rsl = slice(toff + s * 512, toff + (s + 1) * 512)
                            tsl = slice(s * 512, (s + 1) * 512)
                            nc.tensor.matmul(
                                pl1[:, sl],
                                lhsT=wa_t[:, c * P:(c + 1) * P],
                                rhs=at[:, tsl],
                                start=True, stop=False)
                            nc.tensor.matmul(
                                pl1[:, sl],
                                lhsT=wb_t[:, c * P:(c + 1) * P],
                                rhs=ea_t[:, rsl],
                                start=False, stop=True)

                    z = sb.tile([P, 2 * TILE_E], bf16, tag="z")
                    nc.scalar.activation(z[:], pl1[:], AF.Prelu,
                                         bias=0.0, scale=1.0, alpha=LEAKY)

                    py = psY.tile([P, TILE_E], mybir.dt.float32, tag="y")
                    for s in range(2):
                        sl = slice(s * 512, (s + 1) * 512)
                        for c in range(2):
                            nc.tensor.matmul(
                                py[0:D, sl],
                                lhsT=w2_t[:, c * D:(c + 1) * D],
                                rhs=z[:, c * TILE_E + s * 512:
                                      c * TILE_E + (s + 1) * 512],
                                start=(c == 0), stop=(c == 1))

                    yb = sb.tile([D, TILE_E], bf16, tag="yb")
                    nc.vector.tensor_scalar_add(yb[:], py[0:D, :], b2col)
                    sq = sb.tile([D, TILE_E], bf16, tag="sq")
                    nc.vector.scalar_tensor_tensor(
                        out=sq[:], in0=yb[:], scalar=1.0,
                        in1=yb[:], op0=OP.mult, op1=OP.mult)
                    for s in range(2):
                        sl = slice(s * 512, (s + 1) * 512)
                        nc.tensor.matmul(py[D:P, sl], lhsT=on_t[:],
                                         rhs=sq[:, sl],
                                         start=True, stop=True)

                    rsq = sb.tile([D, TILE_E], bf16, tag="rsq")
                    nc.scalar.activation(rsq[:], py[D:P, :],
                                         AF.Abs_reciprocal_sqrt,
                                         bias=bia, scale=scl)
                    ot = ob.tile([D, TILE_E], bf16, tag="ot")
                    nc.vector.scalar_tensor_tensor(
                        out=ot[:], in0=yb[:], scalar=1.0,
                        in1=rsq[:], op0=OP.mult, op1=OP.mult)
                    nc.sync.dma_start(outT[:, e0:e0 + TILE_E], ot[:])

    if not nc.is_finalized():
        nc.finalize()
    return nc


def _pack_hilo_rows(x):
    """[rows, 64] f32 -> [rows, 128] bf16 (hi | lo) row layout."""
    x = np.asarray(x, np.float32)
    hi = x.astype(BF)
    lo = (x - hi.astype(np.float32)).astype(BF)
    return np.ascontiguousarray(np.concatenate([hi, lo], axis=1))


def _wrap16(v):
    """[n] -> [16, n//16]: idx i at (i%16, i//16)."""
    return np.ascontiguousarray(v.reshape(-1, 16).T)


def prep_shared(x_u, W1, b1, W2, b2, gamma):
    W1 = np.asarray(W1, np.float32)
    W2 = np.asarray(W2, np.float32)
    b1p = (np.asarray(b1, np.float32)
           + np.asarray(x_u, np.float32) @ W1[192:256])
    gamma = np.asarray(gamma, np.float32)
    w2p = np.empty((P, P), np.float32)
    w2p[:, 0:D] = W2[0:P]
    w2p[:, D:P] = W2[P:MSG]
    cst = np.zeros((P, 3), np.float32)
    cst[0:D, 0] = np.asarray(b2, np.float32)
    cst[D:P, 1] = 1.0 / (D * gamma * gamma)
    cst[D:P, 2] = EPS / (gamma * gamma)
    return {
        "wa": np.ascontiguousarray(W1[0:P].astype(BF)),
        "wb": np.ascontiguousarray(
            np.concatenate([W1[P:P + D], b1p[None, :]], 0).astype(BF)),
        "w2": w2p.astype(BF),
        "ones64": np.ones((D, D), BF),
        "identb": np.eye(P, dtype=BF),
        "cst": cst,
    }


def prep_core(core, eids, src, tgt, ea, xs_half, xt_half, t_groups, shared):
    """eids: int64 edge ids assigned to this core (-1 = pad)."""
    e_pad = t_groups * GROUP
    k = core // 2
    hs, ht = k >> 1, k & 1

    valid = eids >= 0
    eid0 = np.where(valid, eids, 0)
    sv = (src[eid0] - hs * HALF).astype(np.int16)
    tv = (tgt[eid0] - ht * HALF).astype(np.int16)
    sv[~valid] = 0
    tv[~valid] = 0

    sidx = np.empty((t_groups, P, GROUP // 16), np.int16)
    tidx = np.empty((t_groups, P, GROUP // 16), np.int16)
    for g in range(t_groups):
        sidx[g] = np.tile(_wrap16(sv[g * GROUP:(g + 1) * GROUP]), (8, 1))
        tidx[g] = np.tile(_wrap16(tv[g * GROUP:(g + 1) * GROUP]), (8, 1))

    ea_r = np.where(valid[:, None], ea[eid0], 0).astype(np.float32)
    eaT = np.empty((D + 1, e_pad), BF)
    eaT[0:D] = ea_r.T.astype(BF)
    eaT[D] = BF(1.0)

    return {"xsh": xs_half[hs], "xth": xt_half[ht],
            "sidx": sidx, "tidx": tidx, "eaT": eaT, **shared}


def assign_edges(src, tgt):
    """Split edges into 8 per-core id lists by (src-half, tgt-half) class."""
    cls = (src >= HALF).astype(np.int64) * 2 + (tgt >= HALF)
    order = np.argsort(cls, kind="stable")
    counts = np.bincount(cls, minlength=4)
    lists = []
    pos = 0
    for k in range(4):
        chunk = order[pos:pos + counts[k]]
        pos += counts[k]
        n0 = (len(chunk) + 1) // 2
        lists.append(chunk[:n0])
        lists.append(chunk[n0:])
    return lists


_CACHE = {}
TRACE = False
LAST_RESULT = None


def kernel(x_s, x_t, edge_index, edge_attr, x_u, W1, b1, W2, b2, gamma):
    global LAST_RESULT
    from concourse.bass_utils import run_bass_kernel_spmd

    src = np.asarray(edge_index[0], np.int64)
    tgt = np.asarray(edge_index[1], np.int64)
    ea = np.asarray(edge_attr, np.float32)
    x_s = np.asarray(x_s, np.float32)
    x_t = np.asarray(x_t, np.float32)
    e_total = src.shape[0]

    lists = assign_edges(src, tgt)
    n_max = max(len(l) for l in lists)
    t_groups = -(-n_max // GROUP)

    key = t_groups
    if key not in _CACHE:
        _CACHE[key] = build_nc(t_groups)
    nc = _CACHE[key]

    shared = prep_shared(x_u, W1, b1, W2, b2, gamma)
    xs_half = [_pack_hilo_rows(x_s[0:HALF]), _pack_hilo_rows(x_s[HALF:2 * HALF])]
    xt_half = [_pack_hilo_rows(x_t[0:HALF]), _pack_hilo_rows(x_t[HALF:2 * HALF])]

    e_pad = t_groups * GROUP
    in_maps = []
    eids_all = []
    for c in range(N_CORES):
        eids = np.full(e_pad, -1, np.int64)
        eids[:len(lists[c])] = lists[c]
        eids_all.append(eids)
        in_maps.append(
            prep_core(c, eids, src, tgt, ea, xs_half, xt_half,
                      t_groups, shared))

    res = run_bass_kernel_spmd(nc, in_maps, list(range(N_CORES)), trace=TRACE)
    LAST_RESULT = res

    out = np.empty((e_total, D), np.float32)
    for c in range(N_CORES):
        eids = eids_all[c]
        valid = eids >= 0
        out[eids[valid]] = res.results[c]["outT"].T.astype(np.float32)[valid]
    return out
